# revision 5
# baseline (speedup 1.0000x reference)
"""Conv4d (Strang rearrange) Trainium2 kernel.

Reference op: y = conv_general_dilated(x, w, strides=(1,1,2,2),
padding=((1,1),(1,1),(0,0),(0,0)), dims NCUVHW/OIUVHW) + b.
  x: [4, 4, 32, 32, 64, 64] f32   w: [4, 4, 3, 3, 2, 2]   b: [4]
  y: [4, 4, 32, 32, 32, 32] f32

Decomposition: the stride-2 2x2 tail is non-overlapping, so with
i = io*16 + ib (h = 2*i + kh), the conv is 18 shifted matmuls
(ku,kv,kw shifts) with contraction (ci, kh) per output row.  Weights are
block-diagonal over ib (16 blocks of [8 x 4]) so one matmul carries
K = (ci4, kh2, ib16) = 128, M = (co4, ib16) = 64, N = (v8, io2, j32) = 512.
float32r operands stream at 1 col/cycle for N >= 256.

Sharding: 8 cores = (batch 4) x (D1 halves 2); D1 halo comes from a host-side
pad so the program is uniform across cores.  Host pre-permutes each shard into
the SBUF partition layout so every input DMA is one contiguous 2 MB transfer.
"""

from contextlib import ExitStack

import numpy as np

import concourse.bass as bass
import concourse.tile as tile
from concourse import bacc, mybir
from concourse.bass_utils import run_bass_kernel_spmd

F32R = mybir.dt.float32r
F32 = mybir.dt.float32

B, CIN, COUT = 4, 4, 4
D1, D2, H, W = 32, 32, 64, 64
U = 16            # output u-rows per core
R = U + 2         # input u-rows per core (halo)
V = D2
I, J = H // 2, W // 2
IB, IO = 16, 2    # i = io*16 + ib
VB, VBS = 4, 8    # v blocks of 8
NCORES = 8
NSHIFT = 18

# kv=1 group first: the first matmul of every psum tile is never v-clipped,
# so start=True always covers the full 512 columns.
SHIFTS = [(ku, kv, kw) for kv in (1, 0, 2) for ku in range(3) for kw in range(2)]


def _host_weights(w, b):
    wbd = np.zeros((NSHIFT, 128, 64), np.float32)
    w = np.asarray(w, np.float32)
    for s, (ku, kv, kw) in enumerate(SHIFTS):
        for kh in range(2):
            for ib in range(IB):
                # rows: ci*32 + 2*ib + kh ; cols: co*16 + ib
                wbd[s, 2 * ib + kh : 128 : 32, ib:64:16] = w[:, :, ku, kv, kh, kw].T
    wbd_t = np.ascontiguousarray(wbd.transpose(1, 0, 2))  # [128, 18, 64]
    bias = np.repeat(np.asarray(b, np.float32), IB).reshape(64, 1)
    return wbd_t, bias


def _host_shard(x):
    """x [B,CIN,D1,D2,H,W] -> per-core xs [R, 128, V, IO, J, 2] f32."""
    xp = np.pad(np.asarray(x, np.float32), ((0, 0), (0, 0), (1, 1), (0, 0), (0, 0), (0, 0)))
    shards = []
    for core in range(NCORES):
        bb, half = divmod(core, 2)
        xs = xp[bb, :, half * U : half * U + R]        # [CIN, R, V, H, W]
        xs = xs.reshape(CIN, R, V, IO, 32, W)          # h = io*32 + hl
        xs = np.ascontiguousarray(xs.transpose(1, 0, 4, 2, 3, 5))  # [r, ci, hl, v, io, w]
        shards.append(xs.reshape(R, 128, V, IO, J, 2))
    return shards


def _build_program():
    nc = bacc.Bacc("TRN2", target_bir_lowering=False, debug=False)
    xs = nc.dram_tensor("xs", [R, 128, V, IO, J, 2], F32R, kind="ExternalInput").ap()
    wbd = nc.dram_tensor("wbd", [128, NSHIFT, 64], F32R, kind="ExternalInput").ap()
    bias = nc.dram_tensor("bias", [64, 1], F32, kind="ExternalInput").ap()
    ys = nc.dram_tensor("ys", [U, VB, 64, VBS, IO, J], F32, kind="ExternalOutput").ap()

    with tile.TileContext(nc) as tc, ExitStack() as ctx:
        zpool = ctx.enter_context(tc.tile_pool(name="z", bufs=5))
        cpool = ctx.enter_context(tc.tile_pool(name="consts", bufs=1))
        opool = ctx.enter_context(tc.tile_pool(name="out", bufs=6))
        pspool = ctx.enter_context(
            tc.tile_pool(name="ps", bufs=6, space=bass.MemorySpace.PSUM)
        )

        wt = cpool.tile([128, NSHIFT, 64], F32R, tag="wt")
        nc.sync.dma_start(wt[:], wbd[:])
        bt = cpool.tile([64, 1], F32, tag="bt")
        nc.sync.dma_start(bt[:], bias[:])

        zt = {}

        def load_z(r):
            t = zpool.tile([128, V, IO, J, 2], F32R)
            nc.sync.dma_start(t[:], xs[r])
            zt[r] = t

        for r in range(2):
            load_z(r)
        for u in range(U):
            load_z(u + 2)
            for vb in range(VB):
                v0 = vb * VBS
                ps = pspool.tile([64, VBS, IO, J], F32)
                for s, (ku, kv, kw) in enumerate(SHIFTS):
                    vv0 = max(0, 1 - kv - v0)
                    vv1 = min(VBS, V + 1 - kv - v0)
                    a = v0 + vv0 + kv - 1
                    nc.tensor.matmul(
                        ps[:, vv0:vv1, :, :],
                        wt[:, s, :],
                        zt[u + ku][:, a : a + (vv1 - vv0), :, :, kw],
                        start=(s == 0),
                        stop=(s == NSHIFT - 1),
                    )
                ot = opool.tile([64, VBS, IO, J], F32)
                nc.scalar.activation(
                    ot[:], ps[:], mybir.ActivationFunctionType.Identity, bias=bt[:]
                )
                nc.sync.dma_start(ys[u, vb], ot[:])
            zt.pop(u - 1, None)
    nc.compile()
    return nc


def _unshard(results):
    y = np.empty((B, COUT, D1, D2, I, J), np.float32)
    for core in range(NCORES):
        bb, half = divmod(core, 2)
        arr = results[core]["ys"].reshape(U, VB, COUT, IB, VBS, IO, J)
        arr = arr.transpose(2, 0, 1, 4, 5, 3, 6)  # [co, u, vb, vv, io, ib, j]
        y[bb, :, half * U : (half + 1) * U] = arr.reshape(COUT, U, V, I, J)
    return y


TRACE = False
LAST_RESULT = [None]


def kernel(x, w, b, _cache={}):
    if "nc" not in _cache:
        _cache["nc"] = _build_program()
    nc = _cache["nc"]
    wbd_t, bias = _host_weights(w, b)
    in_maps = [
        {"xs": xs, "wbd": wbd_t, "bias": bias} for xs in _host_shard(x)
    ]
    res = run_bass_kernel_spmd(nc, in_maps, list(range(NCORES)), trace=TRACE)
    LAST_RESULT[0] = res
    return _unshard(res.results)


# revision 9
# speedup vs baseline: 3.0275x; 3.0275x over previous
"""Conv4d (Strang rearrange) Trainium2 kernel.

Reference op: y = conv_general_dilated(x, w, strides=(1,1,2,2),
padding=((1,1),(1,1),(0,0),(0,0)), dims NCUVHW/OIUVHW) + b.
  x: [4, 4, 32, 32, 64, 64] f32   w: [4, 4, 3, 3, 2, 2]   b: [4]
  y: [4, 4, 32, 32, 32, 32] f32

Decomposition: the stride-2 2x2 tail is non-overlapping, so with
i = io*16 + ib (h = 2*i + kh), the conv is 18 shifted matmuls
(ku,kv,kw shifts) with contraction (ci, kh) per output row.  Weights are
block-diagonal over ib (16 blocks of [8 x 4]) so one matmul carries
K = (ci4, kh2, ib16) = 128, M = (co4, ib16) = 64, N = (v8, io2, j32) = 512.
Two such matmuls run concurrently in the PE array via column tiling
(out partitions 0:64 / 64:128 -> col groups 0-1 / 2-3), covering two
v-blocks per pass.  Operands are cast to bf16 on-chip (fp32 PSUM accum).

Sharding: 8 cores = (batch 4) x (D1 halves 2); D1 halo comes from a host-side
pad so the program is uniform across cores.  Host pre-permutes each shard into
the SBUF partition layout so every input DMA is one contiguous 2 MB transfer.
"""

from contextlib import ExitStack

import ml_dtypes
import numpy as np

import concourse.bass as bass
import concourse.tile as tile
from concourse import bacc, mybir
from concourse.bass_utils import run_bass_kernel_spmd

BF16 = mybir.dt.bfloat16
F32 = mybir.dt.float32

B, CIN, COUT = 4, 4, 4
D1, D2, H, W = 32, 32, 64, 64
U = 16            # output u-rows per core
R = U + 2         # input u-rows per core (halo)
V = D2
I, J = H // 2, W // 2
IB, IO = 16, 2    # i = io*16 + ib
VB, VBS = 4, 8    # v blocks of 8
NCORES = 8
NSHIFT = 18

# kv=1 group first: the first matmul of every psum group is never v-clipped,
# so start=True always covers the full 512 columns.
SHIFTS = [(ku, kv, kw) for kv in (1, 0, 2) for ku in range(3) for kw in range(2)]


def _host_weights(w, b):
    wbd = np.zeros((NSHIFT, 128, 64), np.float32)
    w = np.asarray(w, np.float32)
    for s, (ku, kv, kw) in enumerate(SHIFTS):
        for kh in range(2):
            for ib in range(IB):
                # rows: ci*32 + 2*ib + kh ; cols: co*16 + ib
                wbd[s, 2 * ib + kh : 128 : 32, ib:64:16] = w[:, :, ku, kv, kh, kw].T
    wbd_t = np.ascontiguousarray(wbd.transpose(1, 0, 2)).astype(ml_dtypes.bfloat16)
    bias = np.tile(np.repeat(np.asarray(b, np.float32), IB), 2).reshape(128, 1)
    return wbd_t, bias


def _host_shard(x):
    """x [B,CIN,D1,D2,H,W] -> per-core xs [R, 128, V, IO, J, 2] f32."""
    xp = np.pad(np.asarray(x, np.float32), ((0, 0), (0, 0), (1, 1), (0, 0), (0, 0), (0, 0)))
    shards = []
    for core in range(NCORES):
        bb, half = divmod(core, 2)
        xs = xp[bb, :, half * U : half * U + R]        # [CIN, R, V, H, W]
        xs = xs.reshape(CIN, R, V, IO, 32, W)          # h = io*32 + hl
        xs = np.ascontiguousarray(xs.transpose(1, 0, 4, 2, 3, 5))  # [r, ci, hl, v, io, w]
        shards.append(xs.reshape(R, 128, V, IO, J, 2))
    return shards


def _build_program():
    nc = bacc.Bacc("TRN2", target_bir_lowering=False, debug=False)
    xs = nc.dram_tensor("xs", [R, 128, V, IO, J, 2], F32, kind="ExternalInput").ap()
    wbd = nc.dram_tensor("wbd", [128, NSHIFT, 64], BF16, kind="ExternalInput").ap()
    bias = nc.dram_tensor("bias", [128, 1], F32, kind="ExternalInput").ap()
    ys = nc.dram_tensor("ys", [U, 2, 128, VBS, IO, J], F32, kind="ExternalOutput").ap()

    with tile.TileContext(nc) as tc, ExitStack() as ctx:
        zpool = ctx.enter_context(tc.tile_pool(name="z", bufs=3))
        z16pool = ctx.enter_context(tc.tile_pool(name="z16", bufs=5))
        cpool = ctx.enter_context(tc.tile_pool(name="consts", bufs=1))
        opool = ctx.enter_context(tc.tile_pool(name="out", bufs=4))
        pspool = ctx.enter_context(
            tc.tile_pool(name="ps", bufs=4, space=bass.MemorySpace.PSUM)
        )

        wt = cpool.tile([128, NSHIFT, 64], BF16, tag="wt")
        nc.sync.dma_start(wt[:], wbd[:])
        bt = cpool.tile([128, 1], F32, tag="bt")
        nc.sync.dma_start(bt[:], bias[:])

        zt = {}

        def load_z(r):
            t = zpool.tile([128, V, IO, J, 2], F32)
            nc.sync.dma_start(t[:], xs[r])
            t16 = z16pool.tile([128, V, IO, J, 2], BF16)
            nc.vector.tensor_copy(t16[:], t[:])
            zt[r] = t16

        for r in range(2):
            load_z(r)
        for u in range(U):
            load_z(u + 2)
            for pv in range(2):  # pair of v-blocks -> col-tiled matmul pair
                ps = pspool.tile([128, VBS, IO, J], F32)
                for s, (ku, kv, kw) in enumerate(SHIFTS):
                    for hf in range(2):
                        v0 = (pv * 2 + hf) * VBS
                        vv0 = max(0, 1 - kv - v0)
                        vv1 = min(VBS, V + 1 - kv - v0)
                        a = v0 + vv0 + kv - 1
                        nc.tensor.matmul(
                            ps[hf * 64 : (hf + 1) * 64, vv0:vv1, :, :],
                            wt[:, s, :],
                            zt[u + ku][:, a : a + (vv1 - vv0), :, :, kw],
                            start=(s == 0),
                            stop=(s == NSHIFT - 1),
                            skip_group_check=True,
                        )
                ot = opool.tile([128, VBS, IO, J], F32)
                nc.scalar.activation(
                    ot[:], ps[:], mybir.ActivationFunctionType.Identity, bias=bt[:]
                )
                nc.sync.dma_start(ys[u, pv], ot[:])
            zt.pop(u - 1, None)
    nc.compile()
    return nc


def _unshard(results):
    y = np.empty((B, COUT, D1, D2, I, J), np.float32)
    for core in range(NCORES):
        bb, half = divmod(core, 2)
        arr = results[core]["ys"].reshape(U, 2, 2, COUT, IB, VBS, IO, J)
        arr = arr.transpose(3, 0, 1, 2, 5, 6, 4, 7)  # [co,u,pv,hf,vv,io,ib,j]
        y[bb, :, half * U : (half + 1) * U] = arr.reshape(COUT, U, V, I, J)
    return y


TRACE = False
LAST_RESULT = [None]


def kernel(x, w, b, _cache={}):
    if "nc" not in _cache:
        _cache["nc"] = _build_program()
    nc = _cache["nc"]
    wbd_t, bias = _host_weights(w, b)
    in_maps = [
        {"xs": xs, "wbd": wbd_t, "bias": bias} for xs in _host_shard(x)
    ]
    res = run_bass_kernel_spmd(nc, in_maps, list(range(NCORES)), trace=TRACE)
    LAST_RESULT[0] = res
    return _unshard(res.results)


# revision 13
# speedup vs baseline: 3.5212x; 1.1631x over previous
"""Conv4d (Strang rearrange) Trainium2 kernel.

Reference op: y = conv_general_dilated(x, w, strides=(1,1,2,2),
padding=((1,1),(1,1),(0,0),(0,0)), dims NCUVHW/OIUVHW) + b.
  x: [4, 4, 32, 32, 64, 64] f32   w: [4, 4, 3, 3, 2, 2]   b: [4]
  y: [4, 4, 32, 32, 32, 32] f32

Decomposition: the stride-2 2x2 tail is non-overlapping, so with
i = io*16 + ib (h = 2*i + kh), the conv is 18 shifted matmuls
(ku,kv,kw shifts) with contraction (ci, kh) per output row.  Weights are
block-diagonal over ib (16 blocks of [8 x 4]) so one matmul carries
K = (ci4, kh2, ib16) = 128, M = (co4, ib16) = 64, N = (v8, io2, j32) = 512.
Two such matmuls run concurrently in the PE array via column tiling
(out partitions 0:64 / 64:128 -> col groups 0-1 / 2-3), covering two
v-blocks per pass.  Operands are cast to bf16 on-chip (fp32 PSUM accum).

Sharding: 8 cores = (batch 4) x (D1 halves 2); D1 halo comes from a host-side
pad so the program is uniform across cores.  Host pre-permutes each shard into
the SBUF partition layout so every input DMA is one contiguous 2 MB transfer.
"""

from contextlib import ExitStack

import ml_dtypes
import numpy as np

import concourse.bass as bass
import concourse.tile as tile
from concourse import bacc, mybir
from concourse.bass_utils import run_bass_kernel_spmd

BF16 = mybir.dt.bfloat16
F32 = mybir.dt.float32

B, CIN, COUT = 4, 4, 4
D1, D2, H, W = 32, 32, 64, 64
U = 16            # output u-rows per core
R = U + 2         # input u-rows per core (halo)
V = D2
I, J = H // 2, W // 2
IB, IO = 16, 2    # i = io*16 + ib
VB, VBS = 4, 8    # v blocks of 8
NCORES = 8
NSHIFT = 18

# kv=1 group first: the first matmul of every psum group is never v-clipped,
# so start=True always covers the full 512 columns.
SHIFTS = [(ku, kv, kw) for kv in (1, 0, 2) for ku in range(3) for kw in range(2)]


def _host_weights(w, b):
    wbd = np.zeros((NSHIFT, 128, 64), np.float32)
    w = np.asarray(w, np.float32)
    for s, (ku, kv, kw) in enumerate(SHIFTS):
        for kh in range(2):
            for ib in range(IB):
                # rows: ci*32 + 2*ib + kh ; cols: co*16 + ib
                wbd[s, 2 * ib + kh : 128 : 32, ib:64:16] = w[:, :, ku, kv, kh, kw].T
    wbd_t = np.ascontiguousarray(wbd.transpose(1, 0, 2)).astype(ml_dtypes.bfloat16)
    bias = np.tile(np.repeat(np.asarray(b, np.float32), IB), 2).reshape(128, 1)
    return wbd_t, bias


def _host_shard(x):
    """x [B,CIN,D1,D2,H,W] -> per-core xs [R, 128, V, IO, J, 2] f32."""
    xp = np.pad(np.asarray(x, np.float32), ((0, 0), (0, 0), (1, 1), (0, 0), (0, 0), (0, 0)))
    shards = []
    for core in range(NCORES):
        bb, half = divmod(core, 2)
        xs = xp[bb, :, half * U : half * U + R]        # [CIN, R, V, H, W]
        xs = xs.reshape(CIN, R, V, IO, 32, W)          # h = io*32 + hl
        xs = xs.transpose(1, 0, 4, 2, 3, 5).astype(ml_dtypes.bfloat16)  # [r, ci, hl, v, io, w]
        shards.append(np.ascontiguousarray(xs).reshape(R, 128, V, IO, J, 2))
    return shards


def _build_program():
    nc = bacc.Bacc("TRN2", target_bir_lowering=False, debug=False)
    xs = nc.dram_tensor("xs", [R, 128, V, IO, J, 2], BF16, kind="ExternalInput").ap()
    wbd = nc.dram_tensor("wbd", [128, NSHIFT, 64], BF16, kind="ExternalInput").ap()
    bias = nc.dram_tensor("bias", [128, 1], F32, kind="ExternalInput").ap()
    ys = nc.dram_tensor("ys", [U, 2, 128, VBS, IO, J], F32, kind="ExternalOutput").ap()

    with tile.TileContext(nc) as tc, ExitStack() as ctx:
        zpool = ctx.enter_context(tc.tile_pool(name="z", bufs=6))
        cpool = ctx.enter_context(tc.tile_pool(name="consts", bufs=1))
        opool = ctx.enter_context(tc.tile_pool(name="out", bufs=4))
        pspool = ctx.enter_context(
            tc.tile_pool(name="ps", bufs=4, space=bass.MemorySpace.PSUM)
        )

        wt = cpool.tile([128, NSHIFT, 64], BF16, tag="wt")
        nc.sync.dma_start(wt[:], wbd[:])
        bt = cpool.tile([128, 1], F32, tag="bt")
        nc.sync.dma_start(bt[:], bias[:])

        zt = {}

        def load_z(r):
            t = zpool.tile([128, V, IO, J, 2], BF16)
            nc.sync.dma_start(t[:], xs[r])
            zt[r] = t

        for r in range(2):
            load_z(r)
        for u in range(U):
            load_z(u + 2)
            for pv in range(2):  # pair of v-blocks -> col-tiled matmul pair
                ps = pspool.tile([128, VBS, IO, J], F32)
                for s, (ku, kv, kw) in enumerate(SHIFTS):
                    for hf in range(2):
                        v0 = (pv * 2 + hf) * VBS
                        vv0 = max(0, 1 - kv - v0)
                        vv1 = min(VBS, V + 1 - kv - v0)
                        a = v0 + vv0 + kv - 1
                        nc.tensor.matmul(
                            ps[hf * 64 : (hf + 1) * 64, vv0:vv1, :, :],
                            wt[:, s, :],
                            zt[u + ku][:, a : a + (vv1 - vv0), :, :, kw],
                            start=(s == 0),
                            stop=(s == NSHIFT - 1),
                            skip_group_check=True,
                        )
                ot = opool.tile([128, VBS, IO, J], F32)
                nc.scalar.activation(
                    ot[:], ps[:], mybir.ActivationFunctionType.Identity, bias=bt[:]
                )
                nc.sync.dma_start(ys[u, pv], ot[:])
            zt.pop(u - 1, None)
    nc.compile()
    return nc


def _unshard(results):
    y = np.empty((B, COUT, D1, D2, I, J), np.float32)
    for core in range(NCORES):
        bb, half = divmod(core, 2)
        arr = results[core]["ys"].reshape(U, 2, 2, COUT, IB, VBS, IO, J)
        arr = arr.transpose(3, 0, 1, 2, 5, 6, 4, 7)  # [co,u,pv,hf,vv,io,ib,j]
        y[bb, :, half * U : (half + 1) * U] = arr.reshape(COUT, U, V, I, J)
    return y


TRACE = False
LAST_RESULT = [None]


def kernel(x, w, b, _cache={}):
    if "nc" not in _cache:
        _cache["nc"] = _build_program()
    nc = _cache["nc"]
    wbd_t, bias = _host_weights(w, b)
    in_maps = [
        {"xs": xs, "wbd": wbd_t, "bias": bias} for xs in _host_shard(x)
    ]
    res = run_bass_kernel_spmd(nc, in_maps, list(range(NCORES)), trace=TRACE)
    LAST_RESULT[0] = res
    return _unshard(res.results)


# revision 15
# speedup vs baseline: 4.8023x; 1.3638x over previous
"""Conv4d (Strang rearrange) Trainium2 kernel.

Reference op: y = conv_general_dilated(x, w, strides=(1,1,2,2),
padding=((1,1),(1,1),(0,0),(0,0)), dims NCUVHW/OIUVHW) + b.
  x: [4, 4, 32, 32, 64, 64] f32   w: [4, 4, 3, 3, 2, 2]   b: [4]
  y: [4, 4, 32, 32, 32, 32] f32

Decomposition: the stride-2 2x2 tail is non-overlapping, so with
i = io*8 + ib (h = 2*i + kh, w = 2*j + kw) the conv becomes 9 shifted
matmuls (ku,kv shifts) whose contraction is (ci, kh, kw) per output row.
Weights are block-diagonal over ib (8 blocks of [16 x 4]):
  K = (ci4, kh2, kw2, ib8) = 128, M = (co4, ib8) = 32,
  N = (v4, io4, j32) = 512.
Four such matmuls run concurrently via PE column tiling (out partitions
32c..32c+32 -> col group c), covering four v-blocks per pass, with fp32
PSUM accumulation.  Operands are bf16 (host-side cast).

Sharding: 8 cores = (batch 4) x (D1 halves 2); D1 halo comes from a host-side
pad so the program is uniform across cores.  Host pre-permutes each shard into
the SBUF partition layout so every input DMA is one contiguous 1 MB transfer.
"""

from contextlib import ExitStack

import ml_dtypes
import numpy as np

import concourse.bass as bass
import concourse.tile as tile
from concourse import bacc, mybir
from concourse.bass_utils import run_bass_kernel_spmd

BF16 = mybir.dt.bfloat16
F32 = mybir.dt.float32

B, CIN, COUT = 4, 4, 4
D1, D2, H, W = 32, 32, 64, 64
U = 16            # output u-rows per core
R = U + 2         # input u-rows per core (halo)
V = D2
I, J = H // 2, W // 2
IB, IO = 8, 4     # i = io*8 + ib
VBS = 4           # v-block size; 8 v-blocks = 2 rounds x 4 col-tiles
NCORES = 8

# kv=1 group first: the first matmul of every psum group is never v-clipped,
# so start=True always covers the full tile.
SHIFTS = [(ku, kv) for kv in (1, 0, 2) for ku in range(3)]
NSHIFT = len(SHIFTS)


def _host_weights(w, b):
    wbd = np.zeros((NSHIFT, 128, 32), np.float32)
    w = np.asarray(w, np.float32)
    for s, (ku, kv) in enumerate(SHIFTS):
        for kh in range(2):
            for kw in range(2):
                for ib in range(IB):
                    # rows: ci*32 + kh*16 + kw*8 + ib ; cols: co*8 + ib
                    wbd[s, kh * 16 + kw * 8 + ib : 128 : 32, ib : 32 : 8] = (
                        w[:, :, ku, kv, kh, kw].T
                    )
    wbd_t = np.ascontiguousarray(wbd.transpose(1, 0, 2)).astype(ml_dtypes.bfloat16)
    bias = np.tile(np.repeat(np.asarray(b, np.float32), IB), 4).reshape(128, 1)
    return wbd_t, bias


def _host_shard(x):
    """x [B,CIN,D1,D2,H,W] -> per-core xs [R, 128, V, IO, J] bf16."""
    xp = np.pad(np.asarray(x, np.float32), ((0, 0), (0, 0), (1, 1), (0, 0), (0, 0), (0, 0)))
    shards = []
    for core in range(NCORES):
        bb, half = divmod(core, 2)
        xs = xp[bb, :, half * U : half * U + R]           # [CIN, R, V, H, W]
        xs = xs.reshape(CIN, R, V, IO, IB, 2, J, 2)       # h=(io,ib,kh) w=(j,kw)
        xs = xs.transpose(1, 0, 5, 7, 4, 2, 3, 6).astype(ml_dtypes.bfloat16)
        # [r, ci, kh, kw, ib, v, io, j]
        shards.append(np.ascontiguousarray(xs).reshape(R, 128, V, IO, J))
    return shards


def _build_program():
    nc = bacc.Bacc("TRN2", target_bir_lowering=False, debug=False)
    xs = nc.dram_tensor("xs", [R, 128, V, IO, J], BF16, kind="ExternalInput").ap()
    wbd = nc.dram_tensor("wbd", [128, NSHIFT, 32], BF16, kind="ExternalInput").ap()
    bias = nc.dram_tensor("bias", [128, 1], F32, kind="ExternalInput").ap()
    ys = nc.dram_tensor("ys", [U, 2, 128, VBS, IO, J], F32, kind="ExternalOutput").ap()

    with tile.TileContext(nc) as tc, ExitStack() as ctx:
        zpool = ctx.enter_context(tc.tile_pool(name="z", bufs=6))
        cpool = ctx.enter_context(tc.tile_pool(name="consts", bufs=1))
        opool = ctx.enter_context(tc.tile_pool(name="out", bufs=4))
        pspool = ctx.enter_context(
            tc.tile_pool(name="ps", bufs=4, space=bass.MemorySpace.PSUM)
        )

        wt = cpool.tile([128, NSHIFT, 32], BF16, tag="wt")
        nc.sync.dma_start(wt[:], wbd[:])
        bt = cpool.tile([128, 1], F32, tag="bt")
        nc.sync.dma_start(bt[:], bias[:])

        zt = {}

        def load_z(r):
            t = zpool.tile([128, V, IO, J], BF16)
            nc.sync.dma_start(t[:], xs[r])
            zt[r] = t

        for r in range(2):
            load_z(r)
        for u in range(U):
            load_z(u + 2)
            for rnd in range(2):  # 4 col-tiled v-blocks per round
                ps = pspool.tile([128, VBS, IO, J], F32)
                for s, (ku, kv) in enumerate(SHIFTS):
                    for c in range(4):
                        v0 = (rnd * 4 + c) * VBS
                        vv0 = max(0, 1 - kv - v0)
                        vv1 = min(VBS, V + 1 - kv - v0)
                        a = v0 + vv0 + kv - 1
                        nc.tensor.matmul(
                            ps[c * 32 : (c + 1) * 32, vv0:vv1, :, :],
                            wt[:, s, :],
                            zt[u + ku][:, a : a + (vv1 - vv0), :, :],
                            start=(s == 0),
                            stop=(s == NSHIFT - 1),
                            skip_group_check=True,
                            tile_position=(0, c * 32),
                        )
                ot = opool.tile([128, VBS, IO, J], F32)
                nc.scalar.activation(
                    ot[:], ps[:], mybir.ActivationFunctionType.Identity, bias=bt[:]
                )
                nc.sync.dma_start(ys[u, rnd], ot[:])
            zt.pop(u - 1, None)
    nc.compile()
    return nc


def _unshard(results):
    y = np.empty((B, COUT, D1, D2, I, J), np.float32)
    for core in range(NCORES):
        bb, half = divmod(core, 2)
        arr = results[core]["ys"].reshape(U, 2, 4, COUT, IB, VBS, IO, J)
        arr = arr.transpose(3, 0, 1, 2, 5, 6, 4, 7)  # [co,u,rnd,c,vv,io,ib,j]
        y[bb, :, half * U : (half + 1) * U] = arr.reshape(COUT, U, V, I, J)
    return y


TRACE = False
LAST_RESULT = [None]


def kernel(x, w, b, _cache={}):
    if "nc" not in _cache:
        _cache["nc"] = _build_program()
    nc = _cache["nc"]
    wbd_t, bias = _host_weights(w, b)
    in_maps = [
        {"xs": xs, "wbd": wbd_t, "bias": bias} for xs in _host_shard(x)
    ]
    res = run_bass_kernel_spmd(nc, in_maps, list(range(NCORES)), trace=TRACE)
    LAST_RESULT[0] = res
    return _unshard(res.results)


# revision 17
# speedup vs baseline: 5.5219x; 1.1498x over previous
"""Conv4d (Strang rearrange) Trainium2 kernel.

Reference op: y = conv_general_dilated(x, w, strides=(1,1,2,2),
padding=((1,1),(1,1),(0,0),(0,0)), dims NCUVHW/OIUVHW) + b.
  x: [4, 4, 32, 32, 64, 64] f32   w: [4, 4, 3, 3, 2, 2]   b: [4]
  y: [4, 4, 32, 32, 32, 32] f32

Decomposition: the stride-2 2x2 tail is non-overlapping, so with
i = io*8 + ib (h = 2*i + kh, w = 2*j + kw) the conv becomes 9 shifted
matmuls (ku,kv shifts) whose contraction is (ci, kh, kw) per output row.
Weights are block-diagonal over ib (8 blocks of [16 x 4]):
  K = (ci4, kh2, kw2, ib8) = 128, M = (co4, ib8) = 32,
  N = (v4, io4, j32) = 512.
Four such matmuls run concurrently via PE column tiling (out partitions
32c..32c+32 -> col group c), covering four v-blocks per pass, with fp32
PSUM accumulation.  Operands are fp16 (host-side cast); the output is
staged fp16 on-device and upcast to fp32 on the host.

Sharding: 8 cores = (batch 4) x (D1 halves 2); D1 halo comes from a host-side
pad so the program is uniform across cores.  Host pre-permutes each shard into
the SBUF partition layout so every input DMA is one contiguous 1 MB transfer.
"""

from contextlib import ExitStack

import ml_dtypes
import numpy as np

import concourse.bass as bass
import concourse.tile as tile
from concourse import bacc, mybir
from concourse.bass_utils import run_bass_kernel_spmd

F16 = mybir.dt.float16
BF16 = mybir.dt.bfloat16
F32 = mybir.dt.float32

B, CIN, COUT = 4, 4, 4
D1, D2, H, W = 32, 32, 64, 64
U = 16            # output u-rows per core
R = U + 2         # input u-rows per core (halo)
V = D2
I, J = H // 2, W // 2
IB, IO = 8, 4     # i = io*8 + ib
VBS = 4           # v-block size; 8 v-blocks = 2 rounds x 4 col-tiles
NCORES = 8

# kv=1 group first: the first matmul of every psum group is never v-clipped,
# so start=True always covers the full tile.
SHIFTS = [(ku, kv) for kv in (1, 0, 2) for ku in range(3)]
NSHIFT = len(SHIFTS)


def _host_weights(w, b):
    wbd = np.zeros((NSHIFT, 128, 32), np.float32)
    w = np.asarray(w, np.float32)
    for s, (ku, kv) in enumerate(SHIFTS):
        for kh in range(2):
            for kw in range(2):
                for ib in range(IB):
                    # rows: ci*32 + kh*16 + kw*8 + ib ; cols: co*8 + ib
                    wbd[s, kh * 16 + kw * 8 + ib : 128 : 32, ib : 32 : 8] = (
                        w[:, :, ku, kv, kh, kw].T
                    )
    wbd_t = np.ascontiguousarray(wbd.transpose(1, 0, 2)).astype(ml_dtypes.bfloat16)
    bias = np.tile(np.repeat(np.asarray(b, np.float32), IB), 4).reshape(128, 1)
    return wbd_t, bias


def _host_shard(x):
    """x [B,CIN,D1,D2,H,W] -> per-core xs [R, 128, V, IO, J] fp16."""
    xp = np.pad(np.asarray(x, np.float32), ((0, 0), (0, 0), (1, 1), (0, 0), (0, 0), (0, 0)))
    shards = []
    for core in range(NCORES):
        bb, half = divmod(core, 2)
        xs = xp[bb, :, half * U : half * U + R]           # [CIN, R, V, H, W]
        xs = xs.reshape(CIN, R, V, IO, IB, 2, J, 2)       # h=(io,ib,kh) w=(j,kw)
        xs = xs.transpose(1, 0, 5, 7, 4, 2, 3, 6).astype(ml_dtypes.bfloat16)
        # [r, ci, kh, kw, ib, v, io, j]
        shards.append(np.ascontiguousarray(xs).reshape(R, 128, V, IO, J))
    return shards


def _build_program():
    nc = bacc.Bacc("TRN2", target_bir_lowering=False, debug=False)
    xs = nc.dram_tensor("xs", [R, 128, V, IO, J], BF16, kind="ExternalInput").ap()
    wbd = nc.dram_tensor("wbd", [128, NSHIFT, 32], BF16, kind="ExternalInput").ap()
    bias = nc.dram_tensor("bias", [128, 1], F32, kind="ExternalInput").ap()
    ys = nc.dram_tensor("ys", [U, 2, 128, VBS, IO, J], F16, kind="ExternalOutput").ap()

    with tile.TileContext(nc) as tc, ExitStack() as ctx:
        zpool = ctx.enter_context(tc.tile_pool(name="z", bufs=6))
        cpool = ctx.enter_context(tc.tile_pool(name="consts", bufs=1))
        opool = ctx.enter_context(tc.tile_pool(name="out", bufs=4))
        pspool = ctx.enter_context(
            tc.tile_pool(name="ps", bufs=4, space=bass.MemorySpace.PSUM)
        )

        wt = cpool.tile([128, NSHIFT, 32], BF16, tag="wt")
        nc.sync.dma_start(wt[:], wbd[:])
        bt = cpool.tile([128, 1], F32, tag="bt")
        nc.sync.dma_start(bt[:], bias[:])

        zt = {}

        def load_z(r):
            t = zpool.tile([128, V, IO, J], BF16)
            nc.sync.dma_start(t[:], xs[r])
            zt[r] = t

        for r in range(2):
            load_z(r)
        for u in range(U):
            load_z(u + 2)
            for rnd in range(2):  # 4 col-tiled v-blocks per round
                ps = pspool.tile([128, VBS, IO, J], F32)
                for s, (ku, kv) in enumerate(SHIFTS):
                    for c in range(4):
                        v0 = (rnd * 4 + c) * VBS
                        vv0 = max(0, 1 - kv - v0)
                        vv1 = min(VBS, V + 1 - kv - v0)
                        a = v0 + vv0 + kv - 1
                        nc.tensor.matmul(
                            ps[c * 32 : (c + 1) * 32, vv0:vv1, :, :],
                            wt[:, s, :],
                            zt[u + ku][:, a : a + (vv1 - vv0), :, :],
                            start=(s == 0),
                            stop=(s == NSHIFT - 1),
                            skip_group_check=True,
                            tile_position=(0, c * 32),
                        )
                ot = opool.tile([128, VBS, IO, J], F16)
                nc.scalar.activation(
                    ot[:], ps[:], mybir.ActivationFunctionType.Identity, bias=bt[:]
                )
                nc.sync.dma_start(ys[u, rnd], ot[:])
            zt.pop(u - 1, None)
    nc.compile()
    return nc


def _unshard(results):
    y = np.empty((B, COUT, D1, D2, I, J), np.float32)
    for core in range(NCORES):
        bb, half = divmod(core, 2)
        arr = results[core]["ys"].astype(np.float32).reshape(U, 2, 4, COUT, IB, VBS, IO, J)
        arr = arr.transpose(3, 0, 1, 2, 5, 6, 4, 7)  # [co,u,rnd,c,vv,io,ib,j]
        y[bb, :, half * U : (half + 1) * U] = arr.reshape(COUT, U, V, I, J)
    return y


TRACE = False
LAST_RESULT = [None]


def kernel(x, w, b, _cache={}):
    if "nc" not in _cache:
        _cache["nc"] = _build_program()
    nc = _cache["nc"]
    wbd_t, bias = _host_weights(w, b)
    in_maps = [
        {"xs": xs, "wbd": wbd_t, "bias": bias} for xs in _host_shard(x)
    ]
    res = run_bass_kernel_spmd(nc, in_maps, list(range(NCORES)), trace=TRACE)
    LAST_RESULT[0] = res
    return _unshard(res.results)


# revision 18
# speedup vs baseline: 5.8141x; 1.0529x over previous
"""Conv4d (Strang rearrange) Trainium2 kernel.

Reference op: y = conv_general_dilated(x, w, strides=(1,1,2,2),
padding=((1,1),(1,1),(0,0),(0,0)), dims NCUVHW/OIUVHW) + b.
  x: [4, 4, 32, 32, 64, 64] f32   w: [4, 4, 3, 3, 2, 2]   b: [4]
  y: [4, 4, 32, 32, 32, 32] f32

Decomposition: the stride-2 2x2 tail is non-overlapping, so with
i = io*8 + ib (h = 2*i + kh, w = 2*j + kw) the conv becomes 9 shifted
matmuls (ku,kv shifts) whose contraction is (ci, kh, kw) per output row.
Weights are block-diagonal over ib (8 blocks of [16 x 4]):
  K = (ci4, kh2, kw2, ib8) = 128, M = (co4, ib8) = 32,
  N = (v4, io4, j32) = 512.
Four such matmuls run concurrently via PE column tiling (out partitions
32c..32c+32 -> col group c), covering four v-blocks per pass, with fp32
PSUM accumulation.  Operands are fp16 (host-side cast); the output is
staged fp16 on-device and upcast to fp32 on the host.

Sharding: 8 cores = (batch 4) x (D1 halves 2); D1 halo comes from a host-side
pad so the program is uniform across cores.  Host pre-permutes each shard into
the SBUF partition layout so every input DMA is one contiguous 1 MB transfer.
"""

from contextlib import ExitStack

import ml_dtypes
import numpy as np

import concourse.bass as bass
import concourse.tile as tile
from concourse import bacc, mybir
from concourse.bass_utils import run_bass_kernel_spmd

F16 = mybir.dt.float16
BF16 = mybir.dt.bfloat16
F32 = mybir.dt.float32

B, CIN, COUT = 4, 4, 4
D1, D2, H, W = 32, 32, 64, 64
U = 16            # output u-rows per core
R = U + 2         # input u-rows per core (halo)
V = D2
I, J = H // 2, W // 2
IB, IO = 8, 4     # i = io*8 + ib
VBS = 4           # v-block size; 8 v-blocks = 2 rounds x 4 col-tiles
NCORES = 8

# kv=1 group first: the first matmul of every psum group is never v-clipped,
# so start=True always covers the full tile.
SHIFTS = [(ku, kv) for kv in (1, 0, 2) for ku in range(3)]
NSHIFT = len(SHIFTS)


def _host_weights(w, b):
    wbd = np.zeros((NSHIFT, 128, 32), np.float32)
    w = np.asarray(w, np.float32)
    for s, (ku, kv) in enumerate(SHIFTS):
        for kh in range(2):
            for kw in range(2):
                for ib in range(IB):
                    # rows: ci*32 + kh*16 + kw*8 + ib ; cols: co*8 + ib
                    wbd[s, kh * 16 + kw * 8 + ib : 128 : 32, ib : 32 : 8] = (
                        w[:, :, ku, kv, kh, kw].T
                    )
    wbd_t = np.ascontiguousarray(wbd.transpose(1, 0, 2)).astype(ml_dtypes.bfloat16)
    bias = np.tile(np.repeat(np.asarray(b, np.float32), IB), 4).reshape(128, 1)
    return wbd_t, bias


def _host_shard(x):
    """x [B,CIN,D1,D2,H,W] -> per-core xs [R, 128, V, IO, J] fp16."""
    xp = np.pad(np.asarray(x, np.float32), ((0, 0), (0, 0), (1, 1), (0, 0), (0, 0), (0, 0)))
    shards = []
    for core in range(NCORES):
        bb, half = divmod(core, 2)
        xs = xp[bb, :, half * U : half * U + R]           # [CIN, R, V, H, W]
        xs = xs.reshape(CIN, R, V, IO, IB, 2, J, 2)       # h=(io,ib,kh) w=(j,kw)
        xs = xs.transpose(1, 0, 5, 7, 4, 2, 3, 6).astype(ml_dtypes.bfloat16)
        # [r, ci, kh, kw, ib, v, io, j]
        shards.append(np.ascontiguousarray(xs).reshape(R, 128, V, IO, J))
    return shards


def _build_program():
    nc = bacc.Bacc("TRN2", target_bir_lowering=False, debug=False)
    xs = nc.dram_tensor("xs", [R, 128, V, IO, J], BF16, kind="ExternalInput").ap()
    wbd = nc.dram_tensor("wbd", [128, NSHIFT, 32], BF16, kind="ExternalInput").ap()
    bias = nc.dram_tensor("bias", [128, 1], F32, kind="ExternalInput").ap()
    ys = nc.dram_tensor("ys", [U, 2, 128, VBS, IO, J], F16, kind="ExternalOutput").ap()

    with tile.TileContext(nc) as tc, ExitStack() as ctx:
        zpool = ctx.enter_context(tc.tile_pool(name="z", bufs=8))
        cpool = ctx.enter_context(tc.tile_pool(name="consts", bufs=1))
        opool = ctx.enter_context(tc.tile_pool(name="out", bufs=6))
        pspool = ctx.enter_context(
            tc.tile_pool(name="ps", bufs=6, space=bass.MemorySpace.PSUM)
        )

        wt = cpool.tile([128, NSHIFT, 32], BF16, tag="wt")
        nc.sync.dma_start(wt[:], wbd[:])
        bt = cpool.tile([128, 1], F32, tag="bt")
        nc.sync.dma_start(bt[:], bias[:])

        zt = {}

        def load_z(r):
            t = zpool.tile([128, V, IO, J], BF16)
            nc.sync.dma_start(t[:, 0:17], xs[r, :, 0:17])
            nc.sync.dma_start(t[:, 17:V], xs[r, :, 17:V])
            zt[r] = t

        for r in range(2):
            load_z(r)
        for u in range(U):
            load_z(u + 2)
            for rnd in range(2):  # 4 col-tiled v-blocks per round
                ps = pspool.tile([128, VBS, IO, J], F32)
                for s, (ku, kv) in enumerate(SHIFTS):
                    for c in range(4):
                        v0 = (rnd * 4 + c) * VBS
                        vv0 = max(0, 1 - kv - v0)
                        vv1 = min(VBS, V + 1 - kv - v0)
                        a = v0 + vv0 + kv - 1
                        nc.tensor.matmul(
                            ps[c * 32 : (c + 1) * 32, vv0:vv1, :, :],
                            wt[:, s, :],
                            zt[u + ku][:, a : a + (vv1 - vv0), :, :],
                            start=(s == 0),
                            stop=(s == NSHIFT - 1),
                            skip_group_check=True,
                            tile_position=(0, c * 32),
                        )
                ot = opool.tile([128, VBS, IO, J], F16)
                nc.scalar.activation(
                    ot[:], ps[:], mybir.ActivationFunctionType.Identity, bias=bt[:]
                )
                nc.sync.dma_start(ys[u, rnd], ot[:])
            zt.pop(u - 1, None)
    nc.compile()
    return nc


def _unshard(results):
    y = np.empty((B, COUT, D1, D2, I, J), np.float32)
    for core in range(NCORES):
        bb, half = divmod(core, 2)
        arr = results[core]["ys"].astype(np.float32).reshape(U, 2, 4, COUT, IB, VBS, IO, J)
        arr = arr.transpose(3, 0, 1, 2, 5, 6, 4, 7)  # [co,u,rnd,c,vv,io,ib,j]
        y[bb, :, half * U : (half + 1) * U] = arr.reshape(COUT, U, V, I, J)
    return y


TRACE = False
LAST_RESULT = [None]


def kernel(x, w, b, _cache={}):
    if "nc" not in _cache:
        _cache["nc"] = _build_program()
    nc = _cache["nc"]
    wbd_t, bias = _host_weights(w, b)
    in_maps = [
        {"xs": xs, "wbd": wbd_t, "bias": bias} for xs in _host_shard(x)
    ]
    res = run_bass_kernel_spmd(nc, in_maps, list(range(NCORES)), trace=TRACE)
    LAST_RESULT[0] = res
    return _unshard(res.results)
